# revision 35
# baseline (speedup 1.0000x reference)
"""Trainium2 Bass kernel for nn_Evaluation_78383153152424.

Sharding: 8 cores = 2 batches x 4 D-groups (8 planes each). Zero halo,
zero collectives: the 1x1x1 conv chain is pointwise in (d,h,w) and the
9-neighbor gather is local to each (b,d) HxW plane, which each core holds
in full (H=128 rows = 128 SBUF partitions).

Per-core pipeline (SBUF data fp16, PSUM fp32):
  conv chain as block-diagonal matmuls over 16 row-chunks (chunk = 8 rows
  x 160 cols), K=128 contraction; stage-3 col-tiled (tile_position) so a
  group of planes' sim rows share one PSUM tile -> single-op evacuation.
  The sim plane is stored to a reflect-padded DRAM scratch [136,160] per
  plane; ONE 4-dim DMA per GROUP (HWDGE sync queue) then loads all five
  y-shift variants for all planes with reflected edges in place. x-pads
  are reflected in with two group-wide DVE copies. The two (0,0)-shift
  terms are merged host-side -> 17 products: 8 pair-muls + 1 single on
  DVE/GpSimd (x-column split between the two engines), then a batched
  binary add tree (5 ops, also split). relu1/relu2/evac engine choice is
  static per plane/group to balance ACT vs DVE; HAM warmup matmuls run
  on a dummy tile so the PE power ramp starts before the weights land.
"""

import os
import sys
import functools

import numpy as np

for _p in ("/opt/trn_rl_repo", "/root/.axon_site/_ro/trn_rl_repo"):
    if os.path.isdir(_p) and _p not in sys.path:
        sys.path.append(_p)

import concourse.bass as bass
import concourse.tile as tile
from concourse import bacc, mybir
from concourse.bass_utils import run_bass_kernel_spmd

F16, F32 = mybir.dt.float16, mybir.dt.float32
AF = mybir.ActivationFunctionType
OP = mybir.AluOpType

B, G, D, H, W = 2, 8, 32, 128, 160
DG = 8                       # d-planes per core
NCHUNK, RPC = 16, 8          # chunks per plane, rows per chunk
CHUNK_F = RPC * W            # 1280 chunk-local positions
BLOCKS = [(0, 480), (480, 480), (960, 320)]  # whole rows per block (3,3,2)
XPAD = W + 8                 # 168: x-padded row

GROUPS = [int(c) for c in os.environ.get("K_GROUPS", "1421")]
assert sum(GROUPS) == DG
GMAX = max(GROUPS)
NP = DG
RELU1_ENG = os.environ.get("K_RELU1", "a" * NP)
RELU2_ENG = os.environ.get("K_RELU2", "a" * NP)
EVAC_ENG = os.environ.get("K_EVAC", "vvaa")[:len(GROUPS)].ljust(len(GROUPS), "a")
XS = int(os.environ.get("K_XS", "104"))       # DVE gather col width (rest GpSimd)
WARMUP_MM = int(os.environ.get("K_WARMUP", "8"))
MEGA_Q = os.environ.get("K_MEGA_Q", "sync")   # sync (HWDGE) or gpsimd (SWDGE)

# 17-term gather: 8 (narrow, wide) pairs for s != 4 plus the merged center
# term (host adds wide s=4 weights into narrow slot 13). wall slot s holds
# the wide (stride-4) weights, slot 9+s the narrow (stride-2) weights.
PAIRS = [s for s in range(9) if s != 4]


def _ap(t, extra_off, dims):
    return bass.AP(tensor=t.tensor, offset=t.offset + extra_off, ap=dims)


@functools.lru_cache(maxsize=4)
def _build(zb0, zb1, zbsim, groups, relu1_eng, relu2_eng, evac_eng, xs,
           warmup_mm, mega_q):
    groups = list(groups)
    nc = bacc.Bacc("TRN2", target_bir_lowering=False, debug=False, num_devices=8)

    x_ap = nc.dram_tensor("x", [NCHUNK, G, DG * CHUNK_F], F16,
                          kind="ExternalInput").ap()
    wts_ap = nc.dram_tensor("wts", [128, 544], F16, kind="ExternalInput").ap()
    ow_ap = nc.dram_tensor("ow", [128, 19 * W], F16, kind="ExternalInput").ap()
    bia_ap = nc.dram_tensor("bia", [128, 4], F32, kind="ExternalInput").ap()
    out_ap = nc.dram_tensor("out", [DG, H, W], F16, kind="ExternalOutput").ap()
    scr = nc.dram_tensor("scr", [DG, H + 8, W], F16).ap()

    import contextlib
    with tile.TileContext(nc) as tc, contextlib.ExitStack() as ctx:
        wp = ctx.enter_context(tc.tile_pool(name="wp", bufs=1))
        xp = ctx.enter_context(tc.tile_pool(name="xp", bufs=3))
        hp = ctx.enter_context(tc.tile_pool(name="hp", bufs=2))
        h2p = ctx.enter_context(tc.tile_pool(name="h2p", bufs=3))
        gp = ctx.enter_context(tc.tile_pool(name="gp", bufs=2))
        ps1p = ctx.enter_context(tc.tile_pool(name="ps1p", bufs=2, space="PSUM"))
        ps2p = ctx.enter_context(tc.tile_pool(name="ps2p", bufs=2, space="PSUM"))
        ps3p = ctx.enter_context(tc.tile_pool(name="ps3p", bufs=2, space="PSUM"))

        # ---- HAM warmup on a random-filled tile (zeros don't toggle the
        # PE multipliers, so they never trigger the power ramp). Wide
        # N=512 matmuls bridge the gap until the first x tile lands --
        # any PE idle gap resets the ramp timer.
        warm = wp.tile([128, 512], F16)
        nc.vector.random(warm[:])
        ps_w = ps2p.tile([128, 512], F32, tag="ps2", name="ps_warm")
        for _ in range(warmup_mm):
            nc.tensor.matmul(ps_w[:, 0:512], warm[:, 0:128], warm[:],
                             start=True, stop=True)
        del ps_w

        # ---- weights first, then group-0 x (critical path), then the rest
        wts = wp.tile([128, 544], F16)
        nc.sync.dma_start(out=wts[:], in_=wts_ap[:])
        l1a, l1b = wts[:, 0:128], wts[:, 128:256]
        l2a, l2b = wts[:, 256:384], wts[:, 384:512]
        l3 = wts[:, 512:544]
        xt0 = xp.tile([128, groups[0] * CHUNK_F], F16, tag="x", name="xt0")
        for jj in range(groups[0]):
            nc.sync.dma_start(
                out=xt0[:, jj * CHUNK_F:(jj + 1) * CHUNK_F],
                in_=x_ap[:, :, jj * CHUNK_F:(jj + 1) * CHUNK_F])
        # all remaining x loads issued up front so they never queue behind
        # scratch-store / megaload traffic
        xts = [xt0]
        pp = groups[0]
        for gi in range(1, len(groups)):
            gnn = groups[gi]
            xt = xp.tile([128, gnn * CHUNK_F], F16, tag="x", name=f"xt{gi}")
            nc.sync.dma_start(
                out=xt[:],
                in_=x_ap[:, :, pp * CHUNK_F:(pp + gnn) * CHUNK_F])
            xts.append(xt)
            pp += gnn
        ow = wp.tile([128, 19 * W], F16)
        nc.sync.dma_start(out=ow[:], in_=ow_ap[:])
        offs, wgts = ow[:, 0:18 * W], ow[:, 18 * W:19 * W]
        bia = None
        if not (zb0 and zb1 and zbsim):
            bia = wp.tile([128, 4], F32)
            nc.sync.dma_start(out=bia[:], in_=bia_ap[:])

        wgth = wp.tile([128, W], F16)
        nc.scalar.mul(wgth[:], wgts, 0.5)
        wall = wp.tile([128, 18 * W], F16)
        wgth_b = _ap(wgth, 0, [list(wgth.ap[0]), [0, 18], [1, W]])
        nc.vector.tensor_tensor(
            wall[:, :].rearrange("p (s x) -> p s x", s=18),
            offs.rearrange("p (s x) -> p s x", s=18),
            wgth_b, OP.mult)
        # replicate over d so gather muls get contiguous (2x-mode) reads
        wrep = wp.tile([128, 18 * GMAX * W], F16)
        for dd in range(GMAX):
            nc.gpsimd.dma_start(
                out=_ap(wrep, dd * W, [list(wrep.ap[0]), [GMAX * W, 18], [1, W]]),
                in_=_ap(wall, 0, [list(wall.ap[0]), [W, 18], [1, W]]))

        plane0 = 0
        for grp, gn in enumerate(groups):
            planes = list(range(plane0, plane0 + gn))
            plane0 += gn
            gp_rows = 32 * gn

            xt = xts[grp]
            simflat = gp.tile([gp_rows, CHUNK_F], F16, tag="simflat")
            ev_v = evac_eng[grp] == "v"

            # ---- conv chain, block-major: conv3 output is evacuated per
            # block so ps3 is a small [*,512] double-buffered pool (2
            # banks), which frees a bank for ps2 double-buffering.
            for k, (fo, fn) in enumerate(BLOCKS):
                ps3 = ps3p.tile([gp_rows, 512], F32, tag="ps3")
                for j, p in enumerate(planes):
                    xv = xt[:, j * CHUNK_F:(j + 1) * CHUNK_F]
                    # c1b lands at col 512 (bank-aligned; a matmul output
                    # must not straddle a PSUM bank); relu1 spans the gap
                    ps1 = ps1p.tile([128, 1024], F32, tag="ps1")
                    nc.tensor.matmul(ps1[:, 0:fn], l1a, xv[:, fo:fo + fn],
                                     start=True, stop=True)
                    nc.tensor.matmul(ps1[:, 512:512 + fn], l1b,
                                     xv[:, fo:fo + fn], start=True, stop=True)
                    h1 = hp.tile([128, 1024], F16, tag="h1")
                    eng_v1 = relu1_eng[p] == "v"
                    if zb0:
                        if eng_v1:
                            nc.vector.tensor_scalar_max(h1[:, 0:512 + fn],
                                                        ps1[:, 0:512 + fn], 0.0)
                        else:
                            nc.scalar.activation(h1[:, 0:512 + fn],
                                                 ps1[:, 0:512 + fn], AF.Relu)
                    else:
                        for half in (0, 1):
                            sl = slice(half * 512, half * 512 + fn)
                            bb = bia[:, half:half + 1]
                            if eng_v1:
                                nc.vector.tensor_scalar(h1[:, sl], ps1[:, sl],
                                                        bb, 0.0, OP.add, OP.max)
                            else:
                                nc.scalar.activation(h1[:, sl], ps1[:, sl],
                                                     AF.Relu, bias=bb)
                    ps2 = ps2p.tile([128, 512], F32, tag="ps2")
                    nc.tensor.matmul(ps2[:, 0:fn], l2a, h1[:, 0:fn],
                                     start=True, stop=False)
                    nc.tensor.matmul(ps2[:, 0:fn], l2b, h1[:, 512:512 + fn],
                                     start=False, stop=True)
                    h2 = h2p.tile([128, 512], F16, tag="h2")
                    eng_v2 = relu2_eng[p] == "v"
                    if zb1:
                        if eng_v2:
                            nc.vector.tensor_scalar_max(h2[:, 0:fn],
                                                        ps2[:, 0:fn], 0.0)
                        else:
                            nc.scalar.activation(h2[:, 0:fn], ps2[:, 0:fn],
                                                 AF.Relu)
                    else:
                        if eng_v2:
                            nc.vector.tensor_scalar(h2[:, 0:fn], ps2[:, 0:fn],
                                                    bia[:, 2:3], 0.0,
                                                    OP.add, OP.max)
                        else:
                            nc.scalar.activation(h2[:, 0:fn], ps2[:, 0:fn],
                                                 AF.Relu, bias=bia[:, 2:3])
                    nc.tensor.matmul(ps3[32 * j:32 * j + 32, 0:fn],
                                     l3, h2[:, 0:fn], start=True, stop=True,
                                     tile_position=(0, 32 * j))
                # per-block sim evacuation (fp16) + per-block scr store:
                # block k holds complete rows [3k, 3k+nq) of every chunk,
                # so the DRAM roundtrip starts before the group finishes
                so = fo
                if zbsim:
                    if ev_v:
                        nc.vector.tensor_copy(simflat[:, so:so + fn],
                                              ps3[:, 0:fn])
                    else:
                        nc.scalar.copy(simflat[:, so:so + fn], ps3[:, 0:fn])
                elif ev_v:
                    nc.vector.tensor_scalar_add(simflat[:, so:so + fn],
                                                ps3[:, 0:fn],
                                                bia[0:gp_rows, 3:4])
                else:
                    nc.scalar.activation(simflat[:, so:so + fn], ps3[:, 0:fn],
                                         AF.Identity, bias=bia[0:gp_rows, 3:4])
                if os.environ.get("K_BLKSTORE", "1") == "1":
                    nq = fn // W
                    for j, p in enumerate(planes):
                        blk = simflat[32 * j:32 * j + 16, so:so + fn]
                        nc.sync.dma_start(
                            out=_ap(scr[p, 0:1, 0:1], (4 + 3 * k) * W,
                                    [[8 * W, 16], [W, nq], [1, W]]),
                            in_=blk.rearrange("c (q x) -> c q x", x=W))

            # top/bot reflect pads (gpsimd) + megaload; per-plane x reflect
            # pads on DVE (3-dim APs stay on the fast path).
            p0 = planes[0]
            yvall = gp.tile([128, 5 * gn * XPAD], F16, tag="yvall")
            for j, p in enumerate(planes):
                sf = simflat[32 * j:32 * j + 16, :]
                if os.environ.get("K_BLKSTORE", "1") != "1":
                    nc.sync.dma_start(
                        out=scr[p, 4:132, :],
                        in_=sf.rearrange("c (r x) -> c r x", x=W))
                top = sf[0:1, :].rearrange("o (r x) -> o r x", x=W)
                nc.gpsimd.dma_start(out=scr[p, 0:4, :], in_=top[:, 4:0:-1, :])
                bot = sf[15:16, :].rearrange("o (r x) -> o r x", x=W)
                nc.gpsimd.dma_start(out=scr[p, 132:136, :], in_=bot[:, 6:2:-1, :])
                mega_dst = _ap(yvall, j * XPAD + 4,
                               [list(yvall.ap[0]), [gn * XPAD, 5], [1, W]])
                mega_src = _ap(scr[p, 0:1, 0:1], 0,
                               [[W, 128], [2 * W, 5], [1, W]])
                if mega_q == "sync":
                    nc.sync.dma_start(out=mega_dst, in_=mega_src)
                else:
                    nc.gpsimd.dma_start(out=mega_dst, in_=mega_src)
                lp_d = _ap(yvall, j * XPAD,
                           [list(yvall.ap[0]), [gn * XPAD, 5], [1, 4]])
                lp_s = _ap(yvall, j * XPAD + 8,
                           [list(yvall.ap[0]), [gn * XPAD, 5], [-1, 4]])
                nc.gpsimd.tensor_copy(lp_d, lp_s)
                rp_d = _ap(yvall, j * XPAD + 164,
                           [list(yvall.ap[0]), [gn * XPAD, 5], [1, 4]])
                rp_s = _ap(yvall, j * XPAD + 162,
                           [list(yvall.ap[0]), [gn * XPAD, 5], [-1, 4]])
                nc.gpsimd.tensor_copy(rp_d, rp_s)

            # ---- gather: 8 pair-muls + 1 single write 17 slices of P (DVE,
            # dense dsts keep the 2x fp16 mode); the 16->8->4->2->1->+center
            # add tree runs as SBUF->SBUF CCE-accumulate DMAs for all but
            # the last group (zero engine time), on DVE for the tail group.
            gw = gn * W
            P = gp.tile([128, 17 * gw], F16, tag="gtmp")

            for i, s in enumerate(PAIRS):
                iy, ix = s // 3, s % 3
                offA = (iy + 1) * gn * XPAD + 4 + 2 * (ix - 1)
                offB = (2 * iy) * gn * XPAD + 4 + 4 * (ix - 1)
                srcp = _ap(yvall, offA,
                           [list(yvall.ap[0]), [offB - offA, 2],
                            [XPAD, gn], [1, W]])
                w_b = _ap(wrep, (9 + s) * GMAX * W,
                          [list(wrep.ap[0]), [-9 * GMAX * W, 2],
                           [W, gn], [1, W]])
                dst = P[:, 2 * i * gw:(2 * i + 2) * gw].rearrange(
                    "p (t d x) -> p t d x", t=2, d=gn)
                nc.vector.tensor_tensor(dst, w_b, srcp, OP.mult)
            # merged center term -> slice 16
            srcc = _ap(yvall, 2 * gn * XPAD + 4,
                       [list(yvall.ap[0]), [XPAD, gn], [1, W]])
            w_c = _ap(wrep, 13 * GMAX * W,
                      [list(wrep.ap[0]), [W, gn], [1, W]])
            dstc = _ap(P, 16 * gw, [list(P.ap[0]), [W, gn], [1, W]])
            nc.vector.tensor_tensor(dstc, w_c, srcc, OP.mult)

            for width in (8, 4, 2, 1):
                nc.vector.tensor_tensor(P[:, 0:width * gw], P[:, 0:width * gw],
                                        P[:, width * gw:2 * width * gw],
                                        OP.add)
            if grp == len(groups) - 1:
                # tail: final add + output DMA split in chunks so the
                # first columns leave while the rest are still summing
                half = W if gn >= 2 else W // 2
                for ho in range(0, gw, half):
                    nc.vector.tensor_tensor(
                        P[:, ho:ho + half], P[:, ho:ho + half],
                        P[:, 16 * gw + ho:16 * gw + ho + half], OP.add)
                    nc.sync.dma_start(
                        out=_ap(out_ap[p0 + ho // W, 0:1, 0:1], ho % W,
                                [[W, 128], [1, half]]),
                        in_=P[:, ho:ho + half])
            else:
                nc.vector.tensor_tensor(P[:, 0:gw], P[:, 0:gw],
                                        P[:, 16 * gw:17 * gw], OP.add)
                nc.sync.dma_start(
                    out=out_ap[p0:p0 + gn].rearrange("d h x -> h d x"),
                    in_=_ap(P, 0, [list(P.ap[0]), [W, gn], [1, W]]))

    nc.compile()
    return nc


def _pack_weights(w0, bn0_scale, bn0_bias, w1, bn1_scale, bn1_bias, w_sim, b_sim):
    w0f = (w0 * bn0_scale[:, None]).astype(np.float32)
    w1f = (w1 * bn1_scale[:, None]).astype(np.float32)
    l1a = np.zeros((128, 128), np.float16)
    l1b = np.zeros((128, 128), np.float16)
    l2a = np.zeros((128, 128), np.float16)
    l2b = np.zeros((128, 128), np.float16)
    l3 = np.zeros((128, 32), np.float16)
    for c in range(NCHUNK):
        s = slice(c * 8, c * 8 + 8)
        l1a[s, s] = w0f[0:8, :].T
        l1b[s, s] = w0f[8:16, :].T
        l2a[s, s] = w1f[:, 0:8].T
        l2b[s, s] = w1f[:, 8:16].T
        l3[s, c] = w_sim[0, :]
    wts = np.hstack([l1a, l1b, l2a, l2b, l3])
    po = np.arange(128) % 8
    bia = np.stack([bn0_bias[po], bn0_bias[po + 8], bn1_bias[po],
                    np.full(128, float(b_sim[0]))], axis=1).astype(np.float32)
    return wts, bia


def prepare(x1, offset, weight, w0, bn0_scale, bn0_bias, w1, bn1_scale, bn1_bias,
            w_sim, b_sim):
    x1 = np.asarray(x1); offset = np.asarray(offset); weight = np.asarray(weight)
    w0 = np.asarray(w0); bn0_scale = np.asarray(bn0_scale)
    bn0_bias = np.asarray(bn0_bias); w1 = np.asarray(w1)
    bn1_scale = np.asarray(bn1_scale); bn1_bias = np.asarray(bn1_bias)
    w_sim = np.asarray(w_sim); b_sim = np.asarray(b_sim)

    wts, bia = _pack_weights(w0, bn0_scale, bn0_bias, w1, bn1_scale, bn1_bias,
                             w_sim, b_sim)
    zb0 = bool(np.all(bn0_bias == 0))
    zb1 = bool(np.all(bn1_bias == 0))
    zbsim = bool(np.all(b_sim == 0))
    nc = _build(zb0, zb1, zbsim, tuple(GROUPS), RELU1_ENG, RELU2_ENG,
                EVAC_ENG, XS, WARMUP_MM, MEGA_Q)

    in_maps = []
    for core in range(8):
        b, kd = divmod(core, 4)
        # merge the wide (0,0) term (slot 4) into the narrow one (slot 13)
        offm = offset[b].astype(np.float32).copy()
        offm[13] += offm[4]
        ow = np.concatenate([offm.transpose(1, 0, 2).reshape(H, 18 * W),
                             weight[b, 0]], axis=1).astype(np.float16)
        xs = x1[b, :, kd * DG:(kd + 1) * DG].astype(np.float16)
        xs = xs.reshape(G, DG, NCHUNK, RPC, W)
        xs = np.ascontiguousarray(xs.transpose(2, 0, 1, 3, 4)).reshape(
            NCHUNK, G, DG * CHUNK_F)
        in_maps.append({"x": xs, "wts": wts, "ow": ow, "bia": bia})
    return nc, in_maps


def kernel(x1, offset, weight, w0, bn0_scale, bn0_bias, w1, bn1_scale, bn1_bias,
           w_sim, b_sim):
    nc, in_maps = prepare(x1, offset, weight, w0, bn0_scale, bn0_bias, w1,
                          bn1_scale, bn1_bias, w_sim, b_sim)
    res = run_bass_kernel_spmd(nc, in_maps, list(range(8)))
    out = np.empty((B, D, H, W), np.float32)
    for core in range(8):
        b, kd = divmod(core, 4)
        out[b, kd * DG:(kd + 1) * DG] = res.results[core]["out"].astype(
            np.float32)
    return out


# revision 37
# speedup vs baseline: 1.0164x; 1.0164x over previous
"""Trainium2 Bass kernel for nn_Evaluation_78383153152424.

Sharding: 8 cores = 2 batches x 4 D-groups (8 planes each). Zero halo,
zero collectives: the 1x1x1 conv chain is pointwise in (d,h,w) and the
9-neighbor gather is local to each (b,d) HxW plane, which each core holds
in full (H=128 rows = 128 SBUF partitions).

Per-core pipeline (SBUF data fp16, PSUM fp32):
  conv chain as block-diagonal matmuls over 16 row-chunks (chunk = 8 rows
  x 160 cols), K=128 contraction; stage-3 col-tiled (tile_position) so a
  group of planes' sim rows share one PSUM tile -> single-op evacuation.
  The sim plane is stored to a reflect-padded DRAM scratch [136,160] per
  plane; ONE 4-dim DMA per GROUP (HWDGE sync queue) then loads all five
  y-shift variants for all planes with reflected edges in place. x-pads
  are reflected in with two group-wide DVE copies. The two (0,0)-shift
  terms are merged host-side -> 17 products: 8 pair-muls + 1 single on
  DVE/GpSimd (x-column split between the two engines), then a batched
  binary add tree (5 ops, also split). relu1/relu2/evac engine choice is
  static per plane/group to balance ACT vs DVE; HAM warmup matmuls run
  on a dummy tile so the PE power ramp starts before the weights land.
"""

import os
import sys
import functools

import numpy as np

for _p in ("/opt/trn_rl_repo", "/root/.axon_site/_ro/trn_rl_repo"):
    if os.path.isdir(_p) and _p not in sys.path:
        sys.path.append(_p)

import concourse.bass as bass
import concourse.tile as tile
from concourse import bacc, mybir
from concourse.bass_utils import run_bass_kernel_spmd

F16, F32 = mybir.dt.float16, mybir.dt.float32
AF = mybir.ActivationFunctionType
OP = mybir.AluOpType

B, G, D, H, W = 2, 8, 32, 128, 160
DG = 8                       # d-planes per core
NCHUNK, RPC = 16, 8          # chunks per plane, rows per chunk
CHUNK_F = RPC * W            # 1280 chunk-local positions
BLOCKS = [(0, 480), (480, 480), (960, 320)]  # whole rows per block (3,3,2)
XPAD = W + 8                 # 168: x-padded row

GROUPS = [int(c) for c in os.environ.get("K_GROUPS", "1421")]
assert sum(GROUPS) == DG
GMAX = max(GROUPS)
NP = DG
RELU1_ENG = os.environ.get("K_RELU1", "a" * NP)
RELU2_ENG = os.environ.get("K_RELU2", "a" * NP)
EVAC_ENG = os.environ.get("K_EVAC", "vvaa")[:len(GROUPS)].ljust(len(GROUPS), "a")
XS = int(os.environ.get("K_XS", "104"))       # DVE gather col width (rest GpSimd)
WARMUP_MM = int(os.environ.get("K_WARMUP", "8"))
MEGA_Q = os.environ.get("K_MEGA_Q", "sync")   # sync (HWDGE) or gpsimd (SWDGE)

# 17-term gather: 8 (narrow, wide) pairs for s != 4 plus the merged center
# term (host adds wide s=4 weights into narrow slot 13). wall slot s holds
# the wide (stride-4) weights, slot 9+s the narrow (stride-2) weights.
PAIRS = [s for s in range(9) if s != 4]


def _ap(t, extra_off, dims):
    return bass.AP(tensor=t.tensor, offset=t.offset + extra_off, ap=dims)


@functools.lru_cache(maxsize=4)
def _build(zb0, zb1, zbsim, groups, relu1_eng, relu2_eng, evac_eng, xs,
           warmup_mm, mega_q):
    groups = list(groups)
    nc = bacc.Bacc("TRN2", target_bir_lowering=False, debug=False, num_devices=8)

    x_ap = nc.dram_tensor("x", [NCHUNK, G, DG * CHUNK_F], F16,
                          kind="ExternalInput").ap()
    wts_ap = nc.dram_tensor("wts", [128, 544], F16, kind="ExternalInput").ap()
    ow_ap = nc.dram_tensor("ow", [128, 19 * W], F16, kind="ExternalInput").ap()
    bia_ap = nc.dram_tensor("bia", [128, 4], F32, kind="ExternalInput").ap()
    out_ap = nc.dram_tensor("out", [DG, H, W], F16, kind="ExternalOutput").ap()
    scr = nc.dram_tensor("scr", [DG, H + 8, W], F16).ap()

    import contextlib
    with tile.TileContext(nc) as tc, contextlib.ExitStack() as ctx:
        wp = ctx.enter_context(tc.tile_pool(name="wp", bufs=1))
        xp = ctx.enter_context(tc.tile_pool(name="xp", bufs=3))
        hp = ctx.enter_context(tc.tile_pool(name="hp", bufs=2))
        h2p = ctx.enter_context(tc.tile_pool(name="h2p", bufs=3))
        gp = ctx.enter_context(tc.tile_pool(name="gp", bufs=2))
        ps1p = ctx.enter_context(tc.tile_pool(name="ps1p", bufs=2, space="PSUM"))
        ps2p = ctx.enter_context(tc.tile_pool(name="ps2p", bufs=2, space="PSUM"))
        ps3p = ctx.enter_context(tc.tile_pool(name="ps3p", bufs=2, space="PSUM"))

        # ---- HAM warmup on a random-filled tile (zeros don't toggle the
        # PE multipliers, so they never trigger the power ramp). Wide
        # N=512 matmuls bridge the gap until the first x tile lands --
        # any PE idle gap resets the ramp timer.
        warm = wp.tile([128, 512], F16)
        nc.vector.random(warm[:])
        ps_w = ps2p.tile([128, 512], F32, tag="ps2", name="ps_warm")
        for _ in range(warmup_mm):
            nc.tensor.matmul(ps_w[:, 0:512], warm[:, 0:128], warm[:],
                             start=True, stop=True)
        del ps_w

        # ---- weights first, then group-0 x (critical path), then the rest
        wts = wp.tile([128, 544], F16)
        nc.sync.dma_start(out=wts[:], in_=wts_ap[:])
        l1a, l1b = wts[:, 0:128], wts[:, 128:256]
        l2a, l2b = wts[:, 256:384], wts[:, 384:512]
        l3 = wts[:, 512:544]
        xt0 = xp.tile([128, groups[0] * CHUNK_F], F16, tag="x", name="xt0")
        for jj in range(groups[0]):
            nc.sync.dma_start(
                out=xt0[:, jj * CHUNK_F:(jj + 1) * CHUNK_F],
                in_=x_ap[:, :, jj * CHUNK_F:(jj + 1) * CHUNK_F])
        # all remaining x loads issued up front so they never queue behind
        # scratch-store / megaload traffic
        xts = [xt0]
        pp = groups[0]
        for gi in range(1, len(groups)):
            gnn = groups[gi]
            xt = xp.tile([128, gnn * CHUNK_F], F16, tag="x", name=f"xt{gi}")
            nc.sync.dma_start(
                out=xt[:],
                in_=x_ap[:, :, pp * CHUNK_F:(pp + gnn) * CHUNK_F])
            xts.append(xt)
            pp += gnn
        ow = wp.tile([128, 19 * W], F16)
        nc.sync.dma_start(out=ow[:], in_=ow_ap[:])
        offs, wgts = ow[:, 0:18 * W], ow[:, 18 * W:19 * W]
        bia = None
        if not (zb0 and zb1 and zbsim):
            bia = wp.tile([128, 4], F32)
            nc.sync.dma_start(out=bia[:], in_=bia_ap[:])

        wgth = wp.tile([128, W], F16)
        nc.scalar.mul(wgth[:], wgts, 0.5)
        wall = wp.tile([128, 18 * W], F16)
        wgth_b = _ap(wgth, 0, [list(wgth.ap[0]), [0, 18], [1, W]])
        nc.vector.tensor_tensor(
            wall[:, :].rearrange("p (s x) -> p s x", s=18),
            offs.rearrange("p (s x) -> p s x", s=18),
            wgth_b, OP.mult)
        # replicate over d so gather muls get contiguous (2x-mode) reads
        wrep = wp.tile([128, 18 * GMAX * W], F16)
        for dd in range(GMAX):
            nc.gpsimd.dma_start(
                out=_ap(wrep, dd * W, [list(wrep.ap[0]), [GMAX * W, 18], [1, W]]),
                in_=_ap(wall, 0, [list(wall.ap[0]), [W, 18], [1, W]]))

        plane0 = 0
        for grp, gn in enumerate(groups):
            planes = list(range(plane0, plane0 + gn))
            plane0 += gn
            gp_rows = 32 * gn

            xt = xts[grp]
            simflat = gp.tile([gp_rows, CHUNK_F], F16, tag="simflat")
            ev_v = evac_eng[grp] == "v"

            # ---- conv chain, block-major: conv3 output is evacuated per
            # block so ps3 is a small [*,512] double-buffered pool (2
            # banks), which frees a bank for ps2 double-buffering.
            for k, (fo, fn) in enumerate(BLOCKS):
                ps3 = ps3p.tile([gp_rows, 512], F32, tag="ps3")
                for j, p in enumerate(planes):
                    xv = xt[:, j * CHUNK_F:(j + 1) * CHUNK_F]
                    # c1b lands at col 512 (bank-aligned; a matmul output
                    # must not straddle a PSUM bank); relu1 spans the gap
                    ps1 = ps1p.tile([128, 1024], F32, tag="ps1")
                    nc.tensor.matmul(ps1[:, 0:fn], l1a, xv[:, fo:fo + fn],
                                     start=True, stop=True)
                    nc.tensor.matmul(ps1[:, 512:512 + fn], l1b,
                                     xv[:, fo:fo + fn], start=True, stop=True)
                    h1 = hp.tile([128, 1024], F16, tag="h1")
                    eng_v1 = relu1_eng[p] == "v"
                    if zb0:
                        if eng_v1:
                            nc.vector.tensor_scalar_max(h1[:, 0:512 + fn],
                                                        ps1[:, 0:512 + fn], 0.0)
                        else:
                            nc.scalar.activation(h1[:, 0:512 + fn],
                                                 ps1[:, 0:512 + fn], AF.Relu)
                    else:
                        for half in (0, 1):
                            sl = slice(half * 512, half * 512 + fn)
                            bb = bia[:, half:half + 1]
                            if eng_v1:
                                nc.vector.tensor_scalar(h1[:, sl], ps1[:, sl],
                                                        bb, 0.0, OP.add, OP.max)
                            else:
                                nc.scalar.activation(h1[:, sl], ps1[:, sl],
                                                     AF.Relu, bias=bb)
                    ps2 = ps2p.tile([128, 512], F32, tag="ps2")
                    nc.tensor.matmul(ps2[:, 0:fn], l2a, h1[:, 0:fn],
                                     start=True, stop=False)
                    nc.tensor.matmul(ps2[:, 0:fn], l2b, h1[:, 512:512 + fn],
                                     start=False, stop=True)
                    h2 = h2p.tile([128, 512], F16, tag="h2")
                    eng_v2 = relu2_eng[p] == "v"
                    if zb1:
                        if eng_v2:
                            nc.vector.tensor_scalar_max(h2[:, 0:fn],
                                                        ps2[:, 0:fn], 0.0)
                        else:
                            nc.scalar.activation(h2[:, 0:fn], ps2[:, 0:fn],
                                                 AF.Relu)
                    else:
                        if eng_v2:
                            nc.vector.tensor_scalar(h2[:, 0:fn], ps2[:, 0:fn],
                                                    bia[:, 2:3], 0.0,
                                                    OP.add, OP.max)
                        else:
                            nc.scalar.activation(h2[:, 0:fn], ps2[:, 0:fn],
                                                 AF.Relu, bias=bia[:, 2:3])
                    nc.tensor.matmul(ps3[32 * j:32 * j + 32, 0:fn],
                                     l3, h2[:, 0:fn], start=True, stop=True,
                                     tile_position=(0, 32 * j))
                # per-block sim evacuation (fp16) + per-block scr store:
                # block k holds complete rows [3k, 3k+nq) of every chunk,
                # so the DRAM roundtrip starts before the group finishes
                so = fo
                if zbsim:
                    if ev_v:
                        nc.vector.tensor_copy(simflat[:, so:so + fn],
                                              ps3[:, 0:fn])
                    else:
                        nc.scalar.copy(simflat[:, so:so + fn], ps3[:, 0:fn])
                elif ev_v:
                    nc.vector.tensor_scalar_add(simflat[:, so:so + fn],
                                                ps3[:, 0:fn],
                                                bia[0:gp_rows, 3:4])
                else:
                    nc.scalar.activation(simflat[:, so:so + fn], ps3[:, 0:fn],
                                         AF.Identity, bias=bia[0:gp_rows, 3:4])
                if os.environ.get("K_BLKSTORE", "1") == "1":
                    nq = fn // W
                    for j, p in enumerate(planes):
                        blk = simflat[32 * j:32 * j + 16, so:so + fn]
                        nc.sync.dma_start(
                            out=_ap(scr[p, 0:1, 0:1], (4 + 3 * k) * W,
                                    [[8 * W, 16], [W, nq], [1, W]]),
                            in_=blk.rearrange("c (q x) -> c q x", x=W))

            # top/bot reflect pads (gpsimd) + megaload; per-plane x reflect
            # pads on DVE (3-dim APs stay on the fast path).
            p0 = planes[0]
            yvall = gp.tile([128, 5 * gn * XPAD], F16, tag="yvall")
            for j, p in enumerate(planes):
                sf = simflat[32 * j:32 * j + 16, :]
                if os.environ.get("K_BLKSTORE", "1") != "1":
                    nc.sync.dma_start(
                        out=scr[p, 4:132, :],
                        in_=sf.rearrange("c (r x) -> c r x", x=W))
                top = sf[0:1, :].rearrange("o (r x) -> o r x", x=W)
                nc.gpsimd.dma_start(out=scr[p, 0:4, :], in_=top[:, 4:0:-1, :])
                bot = sf[15:16, :].rearrange("o (r x) -> o r x", x=W)
                nc.gpsimd.dma_start(out=scr[p, 132:136, :], in_=bot[:, 6:2:-1, :])
                mega_dst = _ap(yvall, j * XPAD + 4,
                               [list(yvall.ap[0]), [gn * XPAD, 5], [1, W]])
                mega_src = _ap(scr[p, 0:1, 0:1], 0,
                               [[W, 128], [2 * W, 5], [1, W]])
                if mega_q == "sync":
                    nc.sync.dma_start(out=mega_dst, in_=mega_src)
                else:
                    nc.gpsimd.dma_start(out=mega_dst, in_=mega_src)
                lp_d = _ap(yvall, j * XPAD,
                           [list(yvall.ap[0]), [gn * XPAD, 5], [1, 4]])
                lp_s = _ap(yvall, j * XPAD + 8,
                           [list(yvall.ap[0]), [gn * XPAD, 5], [-1, 4]])
                nc.vector.tensor_copy(lp_d, lp_s)
                rp_d = _ap(yvall, j * XPAD + 164,
                           [list(yvall.ap[0]), [gn * XPAD, 5], [1, 4]])
                rp_s = _ap(yvall, j * XPAD + 162,
                           [list(yvall.ap[0]), [gn * XPAD, 5], [-1, 4]])
                nc.vector.tensor_copy(rp_d, rp_s)

            # ---- gather: 8 pair-muls + 1 single write 17 slices of P (DVE,
            # dense dsts keep the 2x fp16 mode); the 16->8->4->2->1->+center
            # add tree runs as SBUF->SBUF CCE-accumulate DMAs for all but
            # the last group (zero engine time), on DVE for the tail group.
            gw = gn * W
            P = gp.tile([128, 17 * gw], F16, tag="gtmp")

            for i, s in enumerate(PAIRS):
                iy, ix = s // 3, s % 3
                offA = (iy + 1) * gn * XPAD + 4 + 2 * (ix - 1)
                offB = (2 * iy) * gn * XPAD + 4 + 4 * (ix - 1)
                srcp = _ap(yvall, offA,
                           [list(yvall.ap[0]), [offB - offA, 2],
                            [XPAD, gn], [1, W]])
                w_b = _ap(wrep, (9 + s) * GMAX * W,
                          [list(wrep.ap[0]), [-9 * GMAX * W, 2],
                           [W, gn], [1, W]])
                dst = P[:, 2 * i * gw:(2 * i + 2) * gw].rearrange(
                    "p (t d x) -> p t d x", t=2, d=gn)
                nc.vector.tensor_tensor(dst, w_b, srcp, OP.mult)
            # merged center term -> slice 16
            srcc = _ap(yvall, 2 * gn * XPAD + 4,
                       [list(yvall.ap[0]), [XPAD, gn], [1, W]])
            w_c = _ap(wrep, 13 * GMAX * W,
                      [list(wrep.ap[0]), [W, gn], [1, W]])
            dstc = _ap(P, 16 * gw, [list(P.ap[0]), [W, gn], [1, W]])
            nc.vector.tensor_tensor(dstc, w_c, srcc, OP.mult)

            for width in (8, 4, 2, 1):
                nc.vector.tensor_tensor(P[:, 0:width * gw], P[:, 0:width * gw],
                                        P[:, width * gw:2 * width * gw],
                                        OP.add)
            if grp == len(groups) - 1:
                # tail: final add + output DMA split in chunks so the
                # first columns leave while the rest are still summing
                half = W if gn >= 2 else W // 2
                for ho in range(0, gw, half):
                    nc.vector.tensor_tensor(
                        P[:, ho:ho + half], P[:, ho:ho + half],
                        P[:, 16 * gw + ho:16 * gw + ho + half], OP.add)
                    nc.sync.dma_start(
                        out=_ap(out_ap[p0 + ho // W, 0:1, 0:1], ho % W,
                                [[W, 128], [1, half]]),
                        in_=P[:, ho:ho + half])
            else:
                nc.vector.tensor_tensor(P[:, 0:gw], P[:, 0:gw],
                                        P[:, 16 * gw:17 * gw], OP.add)
                nc.sync.dma_start(
                    out=out_ap[p0:p0 + gn].rearrange("d h x -> h d x"),
                    in_=_ap(P, 0, [list(P.ap[0]), [W, gn], [1, W]]))

    nc.compile()
    return nc


def _pack_weights(w0, bn0_scale, bn0_bias, w1, bn1_scale, bn1_bias, w_sim, b_sim):
    w0f = (w0 * bn0_scale[:, None]).astype(np.float32)
    w1f = (w1 * bn1_scale[:, None]).astype(np.float32)
    l1a = np.zeros((128, 128), np.float16)
    l1b = np.zeros((128, 128), np.float16)
    l2a = np.zeros((128, 128), np.float16)
    l2b = np.zeros((128, 128), np.float16)
    l3 = np.zeros((128, 32), np.float16)
    for c in range(NCHUNK):
        s = slice(c * 8, c * 8 + 8)
        l1a[s, s] = w0f[0:8, :].T
        l1b[s, s] = w0f[8:16, :].T
        l2a[s, s] = w1f[:, 0:8].T
        l2b[s, s] = w1f[:, 8:16].T
        l3[s, c] = w_sim[0, :]
    wts = np.hstack([l1a, l1b, l2a, l2b, l3])
    po = np.arange(128) % 8
    bia = np.stack([bn0_bias[po], bn0_bias[po + 8], bn1_bias[po],
                    np.full(128, float(b_sim[0]))], axis=1).astype(np.float32)
    return wts, bia


def prepare(x1, offset, weight, w0, bn0_scale, bn0_bias, w1, bn1_scale, bn1_bias,
            w_sim, b_sim):
    x1 = np.asarray(x1); offset = np.asarray(offset); weight = np.asarray(weight)
    w0 = np.asarray(w0); bn0_scale = np.asarray(bn0_scale)
    bn0_bias = np.asarray(bn0_bias); w1 = np.asarray(w1)
    bn1_scale = np.asarray(bn1_scale); bn1_bias = np.asarray(bn1_bias)
    w_sim = np.asarray(w_sim); b_sim = np.asarray(b_sim)

    wts, bia = _pack_weights(w0, bn0_scale, bn0_bias, w1, bn1_scale, bn1_bias,
                             w_sim, b_sim)
    zb0 = bool(np.all(bn0_bias == 0))
    zb1 = bool(np.all(bn1_bias == 0))
    zbsim = bool(np.all(b_sim == 0))
    nc = _build(zb0, zb1, zbsim, tuple(GROUPS), RELU1_ENG, RELU2_ENG,
                EVAC_ENG, XS, WARMUP_MM, MEGA_Q)

    in_maps = []
    for core in range(8):
        b, kd = divmod(core, 4)
        # merge the wide (0,0) term (slot 4) into the narrow one (slot 13)
        offm = offset[b].astype(np.float32).copy()
        offm[13] += offm[4]
        ow = np.concatenate([offm.transpose(1, 0, 2).reshape(H, 18 * W),
                             weight[b, 0]], axis=1).astype(np.float16)
        xs = x1[b, :, kd * DG:(kd + 1) * DG].astype(np.float16)
        xs = xs.reshape(G, DG, NCHUNK, RPC, W)
        xs = np.ascontiguousarray(xs.transpose(2, 0, 1, 3, 4)).reshape(
            NCHUNK, G, DG * CHUNK_F)
        in_maps.append({"x": xs, "wts": wts, "ow": ow, "bia": bia})
    return nc, in_maps


def kernel(x1, offset, weight, w0, bn0_scale, bn0_bias, w1, bn1_scale, bn1_bias,
           w_sim, b_sim):
    nc, in_maps = prepare(x1, offset, weight, w0, bn0_scale, bn0_bias, w1,
                          bn1_scale, bn1_bias, w_sim, b_sim)
    res = run_bass_kernel_spmd(nc, in_maps, list(range(8)))
    out = np.empty((B, D, H, W), np.float32)
    for core in range(8):
        b, kd = divmod(core, 4)
        out[b, kd * DG:(kd + 1) * DG] = res.results[core]["out"].astype(
            np.float32)
    return out
